# revision 1
# baseline (speedup 1.0000x reference)
"""CGConv GNN kernel for trn2, 8-core data-parallel by dst-node range."""
import contextlib
import numpy as np
import ml_dtypes
import concourse.bass as bass
import concourse.bacc as bacc
import concourse.mybir as mybir
import concourse.tile as tile

bf16 = mybir.dt.bfloat16
f32 = mybir.dt.float32
i32 = mybir.dt.int32
AF = mybir.ActivationFunctionType
ALU = mybir.AluOpType

N_NODES, N_EDGES, N_GRAPHS = 100000, 600000, 1000
F_NODE, F_EDGE, H = 12, 6, 128
C = 8
NPC = N_NODES // C            # 12500
NBLK = (NPC + 127) // 128     # 98
NLOC_PAD = NBLK * 128         # 12544


# ---------------------------------------------------------------- host prep
def prep(inputs):
    """Returns (meta, in_maps): per-core input dicts for the SPMD kernel."""
    x = np.asarray(inputs["x"], np.float32)
    ei = np.asarray(inputs["edge_index"]).astype(np.int64)
    ea = np.asarray(inputs["edge_attr"], np.float32)
    batch = np.asarray(inputs["batch"]).astype(np.int64)
    src, dst = ei[0], ei[1]

    order = np.argsort(dst, kind="stable")
    dst_s, src_s = dst[order], src[order]
    ea_s = ea[order]

    core_edges = []
    MAX_CPB = 1
    for c in range(C):
        lo = np.searchsorted(dst_s, c * NPC, "left")
        hi = np.searchsorted(dst_s, (c + 1) * NPC, "left")
        d_l = dst_s[lo:hi] - c * NPC
        blk = d_l // 128
        bc = np.bincount(blk, minlength=NBLK)
        MAX_CPB = max(MAX_CPB, int((bc.max() + 127) // 128))
        core_edges.append((d_l, src_s[lo:hi], ea_s[lo:hi], bc))

    EPB = MAX_CPB * 128
    NCHUNKS = NBLK * MAX_CPB

    W = {k: np.asarray(v, np.float32) for k, v in inputs.items()
         if k not in ("x", "edge_index", "edge_attr", "batch")}

    def cat(a, b):
        return np.concatenate([-a, b], axis=1)  # f-half negated: psum holds [-F | S]

    shared = {
        "W1_a":   cat(W["Wf1"][:F_NODE], W["Ws1"][:F_NODE]).astype(ml_dtypes.bfloat16),
        "W1_mid": cat(W["Wf1"][F_NODE:2*F_NODE], W["Ws1"][F_NODE:2*F_NODE]).astype(ml_dtypes.bfloat16),
        "W1_e":   cat(W["Wf1"][2*F_NODE:], W["Ws1"][2*F_NODE:]).astype(ml_dtypes.bfloat16),
        "b1_rep": np.tile(np.concatenate([-W["bf1"], W["bs1"]])[None, :], (128, 1)).astype(np.float32),
        "Wlin":   W["Wlin"].astype(ml_dtypes.bfloat16),
        "blin":   W["blin"][:, None].astype(np.float32),
        "Wh1":    W["Wh1"].astype(ml_dtypes.bfloat16),
        "bh1_rep": np.tile(W["bh1"][None, :], (128, 1)).astype(np.float32),
        "Wh2":    W["Wh2"].astype(ml_dtypes.bfloat16),
        "bh2_rep": np.tile(W["bh2"][None, :], (128, 1)).astype(np.float32),
        "Wh3":    np.pad(W["Wh3"], ((0, 0), (0, 3))).astype(ml_dtypes.bfloat16),
        "bh3_rep": np.tile(np.pad(W["bh3"], (0, 3))[None, :], (128, 1)).astype(np.float32),
    }
    for L, wf, bf, ws, bs in ((2, "Wf2", "bf2", "Ws2", "bs2"), (3, "Wf3", "bf3", "Ws3", "bs3")):
        shared[f"W{L}_a"] = cat(W[wf][:H], W[ws][:H]).astype(ml_dtypes.bfloat16)
        shared[f"W{L}_mid"] = cat(W[wf][H:2*H], W[ws][H:2*H]).astype(ml_dtypes.bfloat16)
        shared[f"W{L}_e"] = cat(W[wf][2*H:], W[ws][2*H:]).astype(ml_dtypes.bfloat16)
        shared[f"b{L}_rep"] = np.tile(np.concatenate([-W[bf], W[bs]])[None, :], (128, 1)).astype(np.float32)

    in_maps = []
    for c in range(C):
        d_l, s_g, e_c, bc = core_edges[c]
        srcS = np.zeros((NBLK, EPB), np.int32)
        dstB = np.full((NBLK, EPB), -1.0, np.float32)
        eT = np.zeros((F_EDGE, NBLK * EPB), np.float32)
        off = np.zeros(NBLK + 1, np.int64)
        np.cumsum(bc, out=off[1:])
        for b in range(NBLK):
            n = int(bc[b])
            sl = slice(int(off[b]), int(off[b]) + n)
            srcS[b, :n] = s_g[sl]
            dstB[b, :n] = (d_l[sl] - b * 128).astype(np.float32)
            eT[:, b * EPB: b * EPB + n] = e_c[sl].T
        # split-AG remap, 8 segments of each core's node range
        SEG = [0, 13*128, 25*128, 37*128, 49*128, 61*128, 73*128, 85*128, NPC]
        sflat = srcS.reshape(NBLK * MAX_CPB, 128)
        c_of = sflat // NPC
        l_of = sflat % NPC
        srcR = np.zeros_like(sflat)
        offk = 0
        for k in range(len(SEG) - 1):
            n0s, n1s = SEG[k], SEG[k + 1]
            mseg = (l_of >= n0s) & (l_of < n1s)
            srcR[mseg] = (offk + c_of * (n1s - n0s) + (l_of - n0s))[mseg]
            offk += C * (n1s - n0s)
        srcS_pt = np.ascontiguousarray(srcR.astype(np.int32).T)
        dstB_pt = np.ascontiguousarray(dstB.reshape(NBLK * MAX_CPB, 128).T).astype(ml_dtypes.bfloat16)
        x_bf = x.astype(ml_dtypes.bfloat16)
        xsrc = x_bf[srcS.reshape(NBLK * MAX_CPB, 128)]            # [NCHUNKS, 128, 12]
        xsrcT = np.ascontiguousarray(
            xsrc.reshape(NBLK, MAX_CPB, 128, F_NODE).transpose(0, 3, 1, 2).reshape(NBLK, F_NODE, EPB))
        # per-chunk dst replicated down partitions: R[t, n, j] = dst_blocklocal(edge (t, j))
        dstRep = np.ascontiguousarray(
            np.broadcast_to(dstB.reshape(NBLK * MAX_CPB, 1, 128), (NBLK * MAX_CPB, 128, 128))
        ).astype(ml_dtypes.bfloat16)

        xT = np.zeros((F_NODE, NLOC_PAD), np.float32)
        xT[:, :NPC] = x[c * NPC:(c + 1) * NPC].T

        b_loc = batch[c * NPC:(c + 1) * NPC]
        gw0 = int(b_loc.min())
        assert int(b_loc.max()) - gw0 < 128
        batchW = np.full((NBLK, 128), -1.0, np.float32)
        batchW.flat[:NPC] = (b_loc - gw0).astype(np.float32)
        batchW = np.ascontiguousarray(batchW.T).astype(ml_dtypes.bfloat16)
        gidx = np.minimum(gw0 + np.arange(128), 1023).astype(np.int32)[:, None]

        m = dict(shared)
        m.update({
            "srcS": srcS_pt, "dstC": dstB_pt, "dstRep": dstRep, "xsrcT": xsrcT,
            "eT": eT.astype(ml_dtypes.bfloat16),
            "xT": xT, "batchW": batchW, "gidx": gidx,
        })
        in_maps.append(m)

    meta = {"MAX_CPB": MAX_CPB, "EPB": EPB, "NCHUNKS": NCHUNKS}
    return meta, in_maps


# ---------------------------------------------------------------- kernel build
def build(meta):
    MAX_CPB = meta["MAX_CPB"]
    EPB = meta["EPB"]
    NCHUNKS = meta["NCHUNKS"]

    nc = bacc.Bacc("TRN2", target_bir_lowering=False, debug=False, num_devices=C)

    def inp(name, shape, dt):
        return nc.dram_tensor(name, shape, dt, kind="ExternalInput").ap()

    srcS = inp("srcS", [128, NCHUNKS], i32)
    xsrcT = inp("xsrcT", [NBLK, F_NODE, EPB], bf16)
    dstC = inp("dstC", [128, NCHUNKS], bf16)
    dstRep = inp("dstRep", [NCHUNKS, 128, 128], bf16)
    eT = inp("eT", [F_EDGE, NBLK * EPB], bf16)
    xT = inp("xT", [F_NODE, NLOC_PAD], f32)
    batchW = inp("batchW", [128, NBLK], bf16)
    gidx = inp("gidx", [128, 1], i32)
    W1_a = inp("W1_a", [F_NODE, 24], bf16)
    W1_mid = inp("W1_mid", [F_NODE, 24], bf16)
    W1_e = inp("W1_e", [F_EDGE, 24], bf16)
    b1_rep = inp("b1_rep", [128, 24], f32)
    Wlin = inp("Wlin", [F_NODE, H], bf16)
    blin = inp("blin", [H, 1], f32)
    Wmid = {L: inp(f"W{L}_mid", [H, 2 * H], bf16) for L in (2, 3)}
    Wa = {L: inp(f"W{L}_a", [H, 2 * H], bf16) for L in (2, 3)}
    We = {L: inp(f"W{L}_e", [F_EDGE, 2 * H], bf16) for L in (2, 3)}
    brep = {L: inp(f"b{L}_rep", [128, 2 * H], f32) for L in (2, 3)}
    Wh1 = inp("Wh1", [H, H], bf16)
    bh1_rep = inp("bh1_rep", [128, H], f32)
    Wh2 = inp("Wh2", [H, H], bf16)
    bh2_rep = inp("bh2_rep", [128, H], f32)
    Wh3 = inp("Wh3", [H, 4], bf16)
    bh3_rep = inp("bh3_rep", [128, 4], f32)

    out = nc.dram_tensor("out", [1024, 1], f32, kind="ExternalOutput").ap()

    yfs_loc = {L: nc.dram_tensor(f"yfs{L}_loc", [NPC, 2 * H], bf16).ap() for L in (2, 3)}
    yfs_full = {L: nc.dram_tensor(f"yfs{L}_full", [N_NODES, 2 * H], bf16,
                                  addr_space="Shared").ap() for L in (2, 3)}
    pool_in = nc.dram_tensor("pool_in", [1024, H + 4], f32).ap()
    pool_out = nc.dram_tensor("pool_out", [1024, H + 4], f32, addr_space="Shared").ap()

    with tile.TileContext(nc) as tc:
        ctx = contextlib.ExitStack()
        with ctx:
            const = ctx.enter_context(tc.tile_pool(name="const", bufs=1))
            resid = ctx.enter_context(tc.tile_pool(name="resid", bufs=1))
            sb = ctx.enter_context(tc.tile_pool(name="sb", bufs=3))
            gth = ctx.enter_context(tc.tile_pool(name="gth", bufs=6))
            oh = ctx.enter_context(tc.tile_pool(name="oh", bufs=7))
            ps_msg = ctx.enter_context(tc.tile_pool(name="ps_msg", bufs=2, space="PSUM"))
            ps_scat = ctx.enter_context(tc.tile_pool(name="ps_scat", bufs=4, space="PSUM"))
            ps_dense = ctx.enter_context(tc.tile_pool(name="ps_dense", bufs=2, space="PSUM"))


            # ---------------- constants
            iota_i = const.tile([128, 128], i32, tag="iota_i")
            nc.gpsimd.iota(iota_i[:], pattern=[[1, 128]], base=0, channel_multiplier=0)
            iota_row_bf = const.tile([128, 128], bf16, tag="iota_row")
            nc.vector.tensor_copy(out=iota_row_bf[:], in_=iota_i[:])
            iota_ci = const.tile([128, 1], i32, tag="iota_ci")
            nc.gpsimd.iota(iota_ci[:], pattern=[[1, 1]], base=0, channel_multiplier=1)
            iota_colf = const.tile([128, 1], f32, tag="iota_colf")
            nc.vector.tensor_copy(out=iota_colf[:], in_=iota_ci[:])
            ident_bf = const.tile([128, 128], bf16, tag="ident")
            iota_col_bf = const.tile([128, 1], bf16, tag="iota_col_bf")
            nc.vector.tensor_copy(out=iota_col_bf[:], in_=iota_ci[:])
            nc.vector.tensor_tensor(out=ident_bf[:], in0=iota_colf[:].to_broadcast([128, 128]),
                                    in1=iota_row_bf[:], op=ALU.is_equal)

            _cseq = [0]
            def load_const(ap, shape, dt):
                _cseq[0] += 1
                t = const.tile(shape, dt, tag=f"c{_cseq[0]}")
                nc.sync.dma_start(out=t[:], in_=ap[:])
                return t

            W1_a_t = load_const(W1_a, [F_NODE, 24], bf16)
            W1_mid_t = load_const(W1_mid, [F_NODE, 24], bf16)
            W1_e_t = load_const(W1_e, [F_EDGE, 24], bf16)
            b1_rep_t = load_const(b1_rep, [128, 24], f32)
            Wlin_t = load_const(Wlin, [F_NODE, H], bf16)
            blin_t = load_const(blin, [H, 1], f32)
            Wmid_t = {L: load_const(Wmid[L], [H, 2 * H], bf16) for L in (2, 3)}
            Wa_t = {L: load_const(Wa[L], [H, 2 * H], bf16) for L in (2, 3)}
            We_t = {L: load_const(We[L], [F_EDGE, 2 * H], bf16) for L in (2, 3)}
            brep_t = {L: load_const(brep[L], [128, 2 * H], f32) for L in (2, 3)}
            Wh1_t = load_const(Wh1, [H, H], bf16)
            bh1_t = load_const(bh1_rep, [128, H], f32)
            Wh2_t = load_const(Wh2, [H, H], bf16)
            bh2_t = load_const(bh2_rep, [128, H], f32)
            Wh3_t = load_const(Wh3, [H, 4], bf16)
            bh3_t = load_const(bh3_rep, [128, 4], f32)
            srcS_t = load_const(srcS, [128, NCHUNKS], i32)
            dstC_t = load_const(dstC, [128, NCHUNKS], bf16)
            batchW_t = load_const(batchW, [128, NBLK], bf16)
            gidx_t = load_const(gidx, [128, 1], i32)
            xT_t = load_const(xT, [F_NODE, NLOC_PAD], f32)

            hT = resid.tile([128, NLOC_PAD], f32, tag="hT")
            afs_all = resid.tile([128, NBLK, 2 * H], bf16, tag="afs_all")

            SEG = [0, 13*128, 25*128, 37*128, 49*128, 61*128, 73*128, 85*128, NPC]
            SEG_OFF = [0]
            for k in range(len(SEG) - 1):
                SEG_OFF.append(SEG_OFF[-1] + C * (SEG[k + 1] - SEG[k]))

            def seg_ag(L, k):
                nc.gpsimd.collective_compute(
                    "AllGather", ALU.bypass, replica_groups=[list(range(C))],
                    ins=[yfs_loc[L][SEG[k]:SEG[k + 1], :]],
                    outs=[yfs_full[L][SEG_OFF[k]:SEG_OFF[k + 1], :]])

            def onehots(b, t):
                """ohE [edges,128n] bf16, ohT [128n, edges] bf16 for chunk t of block b."""
                tt = b * MAX_CPB + t
                ohE = oh.tile([128, 128], bf16, tag="ohE")
                nc.vector.tensor_tensor(
                    out=ohE[:], in0=dstC_t[:, tt:tt + 1].to_broadcast([128, 128]),
                    in1=iota_row_bf[:], op=ALU.is_equal)
                drt = gth.tile([128, 128], bf16, tag="drt")
                nc.sync.dma_start(out=drt[:], in_=dstRep[tt, :, :])
                ohT = oh.tile([128, 128], bf16, tag="ohT")
                nc.vector.tensor_tensor(
                    out=ohT[:], in0=drt[:],
                    in1=iota_col_bf[:].to_broadcast([128, 128]), op=ALU.is_equal)
                return ohE, ohT, tt

            def dense_yfs(L, b):
                hT_bf = sb.tile([H, 128], bf16, tag="hTbf2")
                nc.vector.tensor_copy(out=hT_bf[:], in_=hT[:, b * 128:(b + 1) * 128])
                ps = ps_dense.tile([128, 2 * H], f32, space="PSUM", tag="dense")
                nc.tensor.matmul(out=ps[:], lhsT=hT_bf[:], rhs=Wmid_t[L][:], start=True, stop=True)
                ysb = sb.tile([128, 2 * H], bf16, tag="ysb")
                nc.vector.tensor_copy(out=ysb[:], in_=ps[:])
                n0, n1 = b * 128, min((b + 1) * 128, NPC)
                if n1 > n0:
                    nc.sync.dma_start(out=yfs_loc[L][n0:n1, :], in_=ysb[:n1 - n0, :])


            # ---------------- LAYER 1 ----------------
            for b in range(NBLK):
                eTb = sb.tile([F_EDGE, EPB], bf16, tag="eTb")
                nc.sync.dma_start(out=eTb[:], in_=eT[:, b * EPB:(b + 1) * EPB])
                xsT_b = sb.tile([F_NODE, EPB], bf16, tag="xsTb")
                nc.sync.dma_start(out=xsT_b[:], in_=xsrcT[b, :, :])
                xT_bf = sb.tile([F_NODE, 128], bf16, tag="xTbf")
                nc.vector.tensor_copy(out=xT_bf[:], in_=xT_t[:, b * 128:(b + 1) * 128])
                afs1_ps = ps_dense.tile([128, 2 * H], f32, space="PSUM", tag="dense")
                nc.tensor.matmul(out=afs1_ps[:, :24], lhsT=xT_bf[:], rhs=W1_a_t[:], start=True, stop=True)
                afs1 = sb.tile([128, 24], bf16, tag="afs1")
                nc.vector.tensor_tensor(out=afs1[:], in0=afs1_ps[:, :24], in1=b1_rep_t[:], op=ALU.add)

                scat = ps_scat.tile([H, 128], f32, space="PSUM", tag="scat")
                for t in range(MAX_CPB):
                    ohE, ohT, tt = onehots(b, t)
                    msg = ps_msg.tile([128, 2 * H], f32, space="PSUM", tag="msg")
                    nc.tensor.matmul(out=msg[:, :24], lhsT=ohT[:], rhs=afs1[:], start=True, stop=False)
                    nc.tensor.matmul(out=msg[:, :24], lhsT=xsT_b[:, t * 128:(t + 1) * 128],
                                     rhs=W1_mid_t[:], start=False, stop=False)
                    nc.tensor.matmul(out=msg[:, :24], lhsT=eTb[:, t * 128:(t + 1) * 128],
                                     rhs=W1_e_t[:], start=False, stop=True)
                    u = sb.tile([128, 24], f32, tag="actu1")
                    nc.scalar.activation(u[:], msg[:, :24], AF.Exp)
                    v = sb.tile([128, 24], f32, tag="actv1")
                    nc.scalar.activation(v[:], u[:], AF.Ln, bias=1.0)
                    sig = sb.tile([128, F_NODE], bf16, tag="sig1")
                    nc.scalar.activation(sig[:], v[:, :F_NODE], AF.Exp, scale=-1.0)
                    m = sb.tile([128, F_NODE], bf16, tag="m1")
                    nc.vector.tensor_tensor(out=m[:], in0=sig[:], in1=v[:, F_NODE:24], op=ALU.mult)
                    nc.tensor.matmul(out=scat[:F_NODE, :], lhsT=m[:], rhs=ohE[:],
                                     start=(t == 0), stop=(t == MAX_CPB - 1))
                h1T_f = sb.tile([F_NODE, 128], f32, tag="h1Tf")
                nc.vector.tensor_tensor(out=h1T_f[:], in0=scat[:F_NODE, :],
                                        in1=xT_t[:, b * 128:(b + 1) * 128], op=ALU.add)
                h1T = sb.tile([F_NODE, 128], bf16, tag="h1T")
                nc.vector.tensor_scalar_max(h1T[:], h1T_f[:], 0.0)
                hT_ps = ps_dense.tile([128, 2 * H], f32, space="PSUM", tag="dense")
                nc.tensor.matmul(out=hT_ps[:, :128], lhsT=Wlin_t[:], rhs=h1T[:], start=True, stop=True)
                nc.vector.tensor_tensor(out=hT[:, b * 128:(b + 1) * 128], in0=hT_ps[:, :128],
                                        in1=blin_t[:].to_broadcast([H, 128]), op=ALU.add)
                dense_yfs(2, b)
                if b in (12, 24, 36, 48, 60, 72, 84):
                    seg_ag(2, {12: 0, 24: 1, 36: 2, 48: 3, 60: 4, 72: 5, 84: 6}[b])

            for b in range(NBLK):
                hT_bf = sb.tile([H, 128], bf16, tag="hTbf")
                nc.vector.tensor_copy(out=hT_bf[:], in_=hT[:, b * 128:(b + 1) * 128])
                afs_ps = ps_dense.tile([128, 2 * H], f32, space="PSUM", tag="dense")
                nc.tensor.matmul(out=afs_ps[:], lhsT=hT_bf[:], rhs=Wa_t[2][:], start=True, stop=True)
                nc.vector.tensor_tensor(out=afs_all[:, b, :], in0=afs_ps[:], in1=brep_t[2][:], op=ALU.add)

            seg_ag(2, 7)

            # ---------------- LAYERS 2, 3 ----------------
            for L in (2, 3):
                for b in range(NBLK):
                    eTb = sb.tile([F_EDGE, EPB], bf16, tag="eTb")
                    nc.sync.dma_start(out=eTb[:], in_=eT[:, b * EPB:(b + 1) * EPB])
                    if L == 2:
                        afs = afs_all[:, b, :]
                    else:
                        hT_bf = sb.tile([H, 128], bf16, tag="hTbf")
                        nc.vector.tensor_copy(out=hT_bf[:], in_=hT[:, b * 128:(b + 1) * 128])
                        afs_ps = ps_dense.tile([128, 2 * H], f32, space="PSUM", tag="dense")
                        nc.tensor.matmul(out=afs_ps[:], lhsT=hT_bf[:], rhs=Wa_t[L][:], start=True, stop=True)
                        afs_t = sb.tile([128, 2 * H], bf16, tag="afs")
                        nc.vector.tensor_tensor(out=afs_t[:], in0=afs_ps[:], in1=brep_t[L][:], op=ALU.add)
                        afs = afs_t[:]

                    scat = ps_scat.tile([H, 128], f32, space="PSUM", tag="scat")
                    for t in range(MAX_CPB):
                        ohE, ohT, tt = onehots(b, t)
                        g = gth.tile([128, 2 * H], bf16, tag="g")
                        nc.gpsimd.indirect_dma_start(
                            out=g[:], out_offset=None, in_=yfs_full[L][:],
                            in_offset=bass.IndirectOffsetOnAxis(ap=srcS_t[:, tt:tt + 1], axis=0))
                        msg = ps_msg.tile([128, 2 * H], f32, space="PSUM", tag="msg")
                        nc.tensor.matmul(out=msg[:], lhsT=ohT[:], rhs=afs, start=True, stop=False)
                        nc.tensor.matmul(out=msg[:], lhsT=ident_bf[:], rhs=g[:], start=False, stop=False)
                        nc.tensor.matmul(out=msg[:], lhsT=eTb[:, t * 128:(t + 1) * 128],
                                         rhs=We_t[L][:], start=False, stop=True)
                        u = sb.tile([128, 2 * H], f32, tag="actu")
                        nc.scalar.activation(u[:], msg[:], AF.Exp)
                        v = sb.tile([128, 2 * H], f32, tag="actv")
                        nc.scalar.activation(v[:], u[:], AF.Ln, bias=1.0)
                        sig = sb.tile([128, H], bf16, tag="sig")
                        nc.scalar.activation(sig[:], v[:, :H], AF.Exp, scale=-1.0)
                        m = sb.tile([128, H], bf16, tag="m")
                        nc.vector.tensor_tensor(out=m[:], in0=sig[:], in1=v[:, H:], op=ALU.mult)
                        nc.tensor.matmul(out=scat[:], lhsT=m[:], rhs=ohE[:],
                                         start=(t == 0), stop=(t == MAX_CPB - 1))
                    htmp = sb.tile([H, 128], f32, tag="htmp")
                    nc.vector.tensor_tensor(out=htmp[:], in0=scat[:], in1=hT[:, b * 128:(b + 1) * 128], op=ALU.add)
                    nc.vector.tensor_scalar_max(hT[:, b * 128:(b + 1) * 128], htmp[:], 0.0)
                    if L == 2:
                        dense_yfs(3, b)
                        if b in (12, 24, 36, 48, 60, 72, 84):
                            seg_ag(3, {12: 0, 24: 1, 36: 2, 48: 3, 60: 4, 72: 5, 84: 6}[b])
                if L == 2:
                    seg_ag(3, 7)

            # ---------------- POOLING ----------------
            zt = sb.tile([128, H + 4], f32, tag="zt")
            nc.vector.memset(zt[:], 0.0)
            for r in range(8):
                nc.sync.dma_start(out=pool_in[r * 128:(r + 1) * 128, :], in_=zt[:])
            pool_ps = ps_scat.tile([128, H + 1], f32, space="PSUM", tag="scat")
            for b in range(NBLK):
                hT_bf = sb.tile([H, 128], bf16, tag="hTbf")
                nc.vector.tensor_copy(out=hT_bf[:], in_=hT[:, b * 128:(b + 1) * 128])
                hblk_ps = ps_dense.tile([128, 128], bf16, space="PSUM", tag="dense")
                nc.tensor.transpose(out=hblk_ps[:], in_=hT_bf[:], identity=ident_bf[:])
                haug = sb.tile([128, H + 1], bf16, tag="haug")
                nc.vector.tensor_copy(out=haug[:, :H], in_=hblk_ps[:])
                nc.vector.memset(haug[:, H:], 1.0)
                ohB = oh.tile([128, 128], bf16, tag="ohE")
                nc.vector.tensor_tensor(
                    out=ohB[:], in0=batchW_t[:, b:b + 1].to_broadcast([128, 128]),
                    in1=iota_row_bf[:], op=ALU.is_equal)
                nc.tensor.matmul(out=pool_ps[:], lhsT=ohB[:], rhs=haug[:],
                                 start=(b == 0), stop=(b == NBLK - 1))
            pool_sb = sb.tile([128, H + 4], f32, tag="poolsb")
            nc.vector.memset(pool_sb[:], 0.0)
            nc.vector.tensor_copy(out=pool_sb[:, :H + 1], in_=pool_ps[:])
            nc.gpsimd.indirect_dma_start(
                out=pool_in[:], out_offset=bass.IndirectOffsetOnAxis(ap=gidx_t[:, :1], axis=0),
                in_=pool_sb[:], in_offset=None)
            nc.gpsimd.collective_compute(
                "AllReduce", ALU.add, replica_groups=[list(range(C))],
                ins=[pool_in[:, :]], outs=[pool_out[:, :]])

            # ---------------- HEAD ----------------
            for r in range(8):
                pt = sb.tile([128, H + 4], f32, tag="pt")
                nc.sync.dma_start(out=pt[:], in_=pool_out[r * 128:(r + 1) * 128, :])
                cnt = sb.tile([128, 1], f32, tag="cnt")
                nc.vector.tensor_scalar_max(cnt[:], pt[:, H:H + 1], 1.0)
                rcnt = sb.tile([128, 1], f32, tag="rcnt")
                nc.vector.reciprocal(rcnt[:], cnt[:])
                gmean = sb.tile([128, H], bf16, tag="gmean")
                nc.vector.tensor_tensor(out=gmean[:], in0=pt[:, :H],
                                        in1=rcnt[:].to_broadcast([128, H]), op=ALU.mult)

                def dense_head(inp_bf, Wt, bt, n_out, relu):
                    tp = ps_dense.tile([128, 128], bf16, space="PSUM", tag="dense")
                    nc.tensor.transpose(out=tp[:], in_=inp_bf[:], identity=ident_bf[:])
                    tT = sb.tile([H, 128], bf16, tag="tT")
                    nc.vector.tensor_copy(out=tT[:], in_=tp[:])
                    ops = ps_dense.tile([128, 2 * H], f32, space="PSUM", tag="dense")
                    nc.tensor.matmul(out=ops[:, :n_out], lhsT=tT[:], rhs=Wt[:], start=True, stop=True)
                    of = sb.tile([128, n_out], f32, tag=f"of{n_out}")
                    nc.vector.tensor_tensor(out=of[:], in0=ops[:, :n_out], in1=bt[:], op=ALU.add)
                    if relu:
                        ob = sb.tile([128, n_out], bf16, tag=f"ob{n_out}")
                        nc.vector.tensor_scalar_max(ob[:], of[:], 0.0)
                        return ob
                    return of

                o1 = dense_head(gmean, Wh1_t, bh1_t, H, True)
                o2 = dense_head(o1, Wh2_t, bh2_t, H, True)
                o3 = dense_head(o2, Wh3_t, bh3_t, 4, False)
                nc.sync.dma_start(out=out[r * 128:(r + 1) * 128, :], in_=o3[:, 0:1])

    import concourse.bacc as _bacc
    _orig_tables = _bacc.get_activation_tables
    def _one_table(arch):
        t = _orig_tables(arch)
        keep = "natural_log_exp_and_others"
        out2 = {}
        for k, v in t.items():
            if k != keep:
                v = {f for f in v if f not in (AF.Exp, AF.Ln)}
            out2[k] = v
        return out2
    _bacc.get_activation_tables = _one_table
    try:
        nc.compile()
    finally:
        _bacc.get_activation_tables = _orig_tables
    return nc


def run(nc, in_maps, trace=False):
    from concourse.bass_utils import run_bass_kernel_spmd
    res = run_bass_kernel_spmd(nc, in_maps, core_ids=list(range(C)), trace=trace)
    return res


# ---------------------------------------------------------------- entry point
_CACHE = {}

def kernel(**inputs):
    """Full-input CGConv GNN on 8 trn2 NeuronCores. Returns [1000, 1] float32."""
    meta, in_maps = prep(inputs)
    key = (meta["MAX_CPB"],)
    if key not in _CACHE:
        _CACHE[key] = build(meta)
    nc = _CACHE[key]
    res = run(nc, in_maps, trace=False)
    return np.asarray(res.results[0]["out"][:N_GRAPHS], dtype=np.float32)

